# revision 11
# baseline (speedup 1.0000x reference)
"""Trainium2 Bass kernel for the dual-GRU-decoder ("Interpolation") problem.

Strategy
--------
Two independent decoders (r: cells 1/2, p: cells 3/4). Each decoder is a
64-step GRU recurrence with B=2048, H=1024, D=128, n1=16.

The per-call wall clock on the axon path is dominated by (a) shipping the
NEFF + per-core inputs through the tunnel and (b) loading the NEFF; device
execution itself is ~10ms. So the kernel is built to minimize BYTES moved
per call:
  * hardware For_i loops over timesteps (static program ~10x smaller than
    full unrolling),
  * bf16 outputs (halves output upload/download; also halves the donated
    zero-buffer upload),
  * NCORES knob: with NCORES=2, each decoder runs on ONE core which
    processes the 2048 batch as 4 sequential slices of 512 — weights are
    then uploaded once per decoder instead of once per core (158MB -> 40MB).

Within a core, all weights are cast to bf16 and kept resident in SBUF
(~154 KiB/partition). Activations live in a transposed layout (feature dim
on partitions, batch on the free dim); the host pre/post-transposes.

Per step and per output chunk i (128 gate channels) the kernel accumulates
r/z gates over the concatenated [x; h] contraction in a single PSUM bank,
keeps the n-gate's input/hidden parts separate (r multiplies only the
hidden part), and applies sigmoid/tanh on the scalar engine with fused
per-partition biases. Hidden state is double-buffered (ping-pong); the
timestep loops are 2-step bodies so the ping-pong stays static.
"""

import time

import numpy as np
import ml_dtypes

BF16 = ml_dtypes.bfloat16
B_FULL, T, D, H, N1 = 2048, 64, 128, 1024, 16
TOUT = T - N1 + 1  # 49
HK = H // 128      # 8 hidden chunks
B = 512            # batch per slice (one matmul free-dim)
P = 128

NCORES = 2         # 2, 4, or 8; G = NCORES//2 cores per decoder
G = NCORES // 2
S = 4 // G         # sequential 512-slices per core
CB = S * B         # batch rows per core

_PROG = None
_TRACE = False
_last = {}


def _build_program():
    import concourse.mybir as mybir
    import concourse.tile as tile
    from concourse import bacc
    from concourse.bass import ds

    f32, bf16 = mybir.dt.float32, mybir.dt.bfloat16
    A = mybir.ActivationFunctionType
    # Bacc (not raw Bass): its compile() pass splits multi-semaphore waits
    # into event-semaphore trees — TRN2 allows at most 1 wait per instruction.
    nc = bacc.Bacc(None, target_bir_lowering=False)

    w1t = nc.dram_tensor("w1t", [9, P, 3 * H], bf16, kind="ExternalInput")
    w2t = nc.dram_tensor("w2t", [16, P, 3 * H], bf16, kind="ExternalInput")
    wot = nc.dram_tensor("wot", [HK, P, P], bf16, kind="ExternalInput")
    wit = nc.dram_tensor("wit", [P, H], bf16, kind="ExternalInput")
    bias = nc.dram_tensor("bias", [P, 73], f32, kind="ExternalInput")
    zt = nc.dram_tensor("zt", [P, S * N1 * B], bf16, kind="ExternalInput")
    z8t = nc.dram_tensor("z8t", [P, S * B], bf16, kind="ExternalInput")
    out_d = nc.dram_tensor("out", [P, S * TOUT * B], bf16, kind="ExternalOutput")

    with tile.TileContext(nc) as tc:
        with (
            tc.tile_pool(name="w", bufs=1) as wpool,
            tc.tile_pool(name="st", bufs=1) as spool,
            tc.tile_pool(name="zin", bufs=2) as zpool,
            tc.tile_pool(name="rz", bufs=2) as rzpool,
            tc.tile_pool(name="tmp", bufs=4) as tpool,
            tc.tile_pool(name="ost", bufs=1) as opool,
            tc.tile_pool(name="psum", bufs=8, space="PSUM") as ppool,
        ):
            # ---- resident weights ----
            w1 = wpool.tile([P, 9, 3 * H], bf16, tag="w1")
            for k in range(9):
                nc.sync.dma_start(w1[:, k, :], w1t[k])
            w2 = wpool.tile([P, 16, 3 * H], bf16, tag="w2")
            for k in range(16):
                nc.sync.dma_start(w2[:, k, :], w2t[k])
            wo = wpool.tile([P, HK, P], bf16, tag="wo")
            nc.sync.dma_start(wo[:], wot.rearrange("o p f -> p o f"))
            witl = wpool.tile([P, H], bf16, tag="wit")
            nc.sync.dma_start(witl[:], wit[:])
            bia = wpool.tile([P, 73], f32, tag="bias")
            nc.sync.dma_start(bia[:], bias[:])
            brz1, bni1, bnh1 = bia[:, 0:16], bia[:, 16:24], bia[:, 24:32]
            brz2, bni2, bnh2 = bia[:, 32:48], bia[:, 48:56], bia[:, 56:64]
            bout, bini = bia[:, 64:65], bia[:, 65:73]

            # ---- state (ping-pong) ----
            h0b = [spool.tile([P, HK, B], bf16, tag=f"h0{i}", name=f"h0{i}")
                   for i in range(2)]
            h1b = [spool.tile([P, HK, B], bf16, tag=f"h1{i}", name=f"h1{i}")
                   for i in range(2)]
            # feedback/output tiles: even steps write otE, odd steps write otO
            otE = spool.tile([P, B], bf16, tag="otE", name="otE")
            otO = spool.tile([P, B], bf16, tag="otO", name="otO")

            # consolidate the many init-DMA queue semaphores into one sync
            # point; otherwise downstream instructions exceed the per-inst
            # sync-wait slot limit in codegen.
            tc.strict_bb_all_engine_barrier()

            def gru_cell(w, rz_ks, in_ks, hn_ks, brz, bni, bnh, h_read, h_write):
                """One GRU cell step, transposed layout.

                rz_ks/in_ks/hn_ks: lists of (w_chunk_index, rhs_ap[128,B])
                pairs for the r/z accumulation, the n-gate input part, and
                the n-gate hidden part respectively.
                """
                for i in range(HK):
                    pr = ppool.tile([P, B], f32, tag="acc")
                    pz = ppool.tile([P, B], f32, tag="acc")
                    phn = ppool.tile([P, B], f32, tag="acc")
                    pin = ppool.tile([P, B], f32, tag="acc")
                    nrz = len(rz_ks)
                    for j, (k, rhs) in enumerate(rz_ks):
                        nc.tensor.matmul(pr[:], w[:, k, ds(i * P, P)], rhs,
                                         start=(j == 0), stop=(j == nrz - 1))
                    for j, (k, rhs) in enumerate(rz_ks):
                        nc.tensor.matmul(pz[:], w[:, k, ds((HK + i) * P, P)], rhs,
                                         start=(j == 0), stop=(j == nrz - 1))
                    for j, (k, rhs) in enumerate(hn_ks):
                        nc.tensor.matmul(phn[:], w[:, k, ds((2 * HK + i) * P, P)], rhs,
                                         start=(j == 0), stop=(j == len(hn_ks) - 1))
                    for j, (k, rhs) in enumerate(in_ks):
                        nc.tensor.matmul(pin[:], w[:, k, ds((2 * HK + i) * P, P)], rhs,
                                         start=(j == 0), stop=(j == len(in_ks) - 1))
                    r = rzpool.tile([P, B], bf16, tag="r")
                    zz = rzpool.tile([P, B], bf16, tag="z")
                    nc.scalar.activation(r[:], pr[:], A.Sigmoid, bias=brz[:, i:i + 1])
                    nc.scalar.activation(zz[:], pz[:], A.Sigmoid,
                                         bias=brz[:, HK + i:HK + i + 1])
                    a = tpool.tile([P, B], f32, tag="tmp")
                    nt = tpool.tile([P, B], f32, tag="tmp")
                    nc.scalar.add(a[:], phn[:], bnh[:, i:i + 1])   # h_n + b_hn
                    nc.vector.tensor_mul(a[:], r[:], a[:])         # r * (...)
                    nc.vector.tensor_add(a[:], a[:], pin[:])       # + i_n
                    nc.scalar.activation(nt[:], a[:], A.Tanh, bias=bni[:, i:i + 1])
                    nc.vector.tensor_sub(a[:], h_read[:, i, :], nt[:])  # h - n
                    nc.vector.tensor_mul(a[:], zz[:], a[:])             # z*(h-n)
                    nc.vector.tensor_add(h_write[:, i, :], nt[:], a[:])  # n + z*(h-n)

            def step(t, xT_ap, out_slice, ot_write, t0=False):
                """One GRU step at parity t%2. xT_ap: [P,B] input AP.
                out_slice: dram AP to store the step's output (or None).
                ot_write: bf16 [P,B] tile to hold the output (or None)."""
                par = t % 2
                h0r, h0w = h0b[par], h0b[1 - par]
                rz1 = [(1 + k, h0r[:, k, :]) for k in range(HK)] + [(0, xT_ap)]
                gru_cell(w1, rz1, [(0, xT_ap)],
                         [(1 + k, h0r[:, k, :]) for k in range(HK)],
                         brz1, bni1, bnh1, h0r, h0w)

                h1r = h0w if t0 else h1b[par]
                h1w = h1b[1 - par]
                rz2 = ([(8 + k, h1r[:, k, :]) for k in range(HK)]
                       + [(k, h0w[:, k, :]) for k in range(HK)])
                gru_cell(w2, rz2, [(k, h0w[:, k, :]) for k in range(HK)],
                         [(8 + k, h1r[:, k, :]) for k in range(HK)],
                         brz2, bni2, bnh2, h1r, h1w)

                if ot_write is not None:
                    po = ppool.tile([P, B], f32, tag="acc")
                    for k in range(HK):
                        nc.tensor.matmul(po[:], wo[:, k, :], h1w[:, k, :],
                                         start=(k == 0), stop=(k == HK - 1))
                    nc.scalar.add(ot_write[:], po[:], bout[:, 0:1])
                    if out_slice is not None:
                        nc.sync.dma_start(out_slice, ot_write[:])

            def load_x(off):
                xT = zpool.tile([P, B], bf16, tag="zin")
                nc.sync.dma_start(xT[:], zt[:, ds(off, B)])
                return xT

            def slice_body(s):
                zoff = s * (N1 * B)
                ooff = s * (TOUT * B)
                # ---- h0 init: h0 = z8 @ w_init.T + b_init ----
                z8l = zpool.tile([P, B], bf16, tag="zin")
                nc.sync.dma_start(z8l[:], z8t[:, ds(s * B, B)])
                for m in range(HK):
                    ps = ppool.tile([P, B], f32, tag="acc")
                    nc.tensor.matmul(ps[:], witl[:, ds(m * P, P)], z8l[:],
                                     start=True, stop=True)
                    nc.scalar.activation(h0b[0][:, m, :], ps[:], A.Identity,
                                         bias=bini[:, m:m + 1])
                # t = 0 unrolled (h1 seeding)
                step(0, load_x(zoff)[:], None, None, t0=True)
                # t = 1..14 input phase (odd, even per body)
                with tc.For_i(0, 7) as i:
                    step(1, load_x(zoff + (i * 2 + 1) * B)[:], None, None)
                    step(0, load_x(zoff + (i * 2 + 2) * B)[:], None, None)
                # t = 15 (emits out j=0 and seeds otO)
                step(1, load_x(zoff + 15 * B)[:], out_d[:, ds(ooff, B)], otO)
                # t = 16..63 feedback phase
                with tc.For_i(8, 32) as i:
                    step(0, otO[:], out_d[:, ds(ooff + (i * 2 - 15) * B, B)], otE)
                    step(1, otE[:], out_d[:, ds(ooff + (i * 2 - 14) * B, B)], otO)

            if S == 1:
                slice_body(0)
            else:
                with tc.For_i(0, S) as s:
                    slice_body(s)

    # Run Bacc's compile passes (register allocation, event-semaphore wait
    # splitting) before the module is serialized for the compiler.
    nc.finalize()
    return nc


def _get_prog():
    global _PROG
    if _PROG is None:
        _PROG = _build_program()
    return _PROG


def _patched_run_via_pjrt(nc, in_maps, n_cores):
    """Drop-in for bass2jax.run_bass_via_pjrt (multi-core, no-debug case)
    with two transfer optimizations:
      * donated zero output buffers are created device-side (jnp.zeros with
        a NamedSharding) instead of being uploaded as 50MB of host zeros;
      * per-core inputs are device_put per device and assembled with
        make_array_from_single_device_arrays (no host-side concat pass).
    Semantics are identical; outputs verified bit-equal to the stock path.
    """
    import jax
    import jax.numpy as jnp
    import numpy as np
    from jax.sharding import Mesh, NamedSharding, PartitionSpec
    from jax.experimental.shard_map import shard_map
    import concourse.mybir as mybir
    import concourse.bass2jax as b2j

    if nc.dbg_addr is not None or n_cores < 2:
        raise RuntimeError("unsupported; use stock path")
    b2j.install_neuronx_cc_hook()
    partition_name = nc.partition_id_tensor.name if nc.partition_id_tensor else None
    in_names, out_names, out_avals, zero_shapes = [], [], [], []
    for alloc in nc.m.functions[0].allocations:
        if not isinstance(alloc, mybir.MemoryLocationSet):
            continue
        name = alloc.memorylocations[0].name
        if alloc.kind == "ExternalInput":
            if name != partition_name:
                in_names.append(name)
        elif alloc.kind == "ExternalOutput":
            shape = tuple(alloc.tensor_shape)
            out_names.append(name)
            out_avals.append(jax.core.ShapedArray(shape, mybir.dt.np(alloc.dtype)))
            zero_shapes.append((shape, mybir.dt.np(alloc.dtype)))
    n_params = len(in_names)
    n_outs = len(out_avals)
    in_names_full = in_names + out_names
    if partition_name is not None:
        in_names_full.append(partition_name)
    donate = tuple(range(n_params, n_params + n_outs))

    def _body(*args):
        operands = list(args)
        if partition_name is not None:
            operands.append(b2j.partition_id_tensor())
        outs = b2j._bass_exec_p.bind(
            *operands,
            out_avals=tuple(out_avals),
            in_names=tuple(in_names_full),
            out_names=tuple(out_names),
            lowering_input_output_aliases=(),
            sim_require_finite=True,
            sim_require_nnan=True,
            nc=nc,
        )
        return tuple(outs)

    devices = jax.devices()[:n_cores]
    assert len(devices) == n_cores
    mesh = Mesh(np.asarray(devices), ("core",))
    in_specs = (PartitionSpec("core"),) * (n_params + n_outs)
    out_specs = (PartitionSpec("core"),) * len(out_names)
    sharded = jax.jit(
        shard_map(_body, mesh=mesh, in_specs=in_specs, out_specs=out_specs,
                  check_rep=False),
        donate_argnums=donate, keep_unused=True,
    )
    sh = NamedSharding(mesh, PartitionSpec("core"))
    per_core = [[np.asarray(m[name]) for name in in_names] for m in in_maps]
    concat_in = []
    for i in range(n_params):
        shards = [jax.device_put(per_core[c][i], devices[c])
                  for c in range(n_cores)]
        gshape = (n_cores * shards[0].shape[0],) + shards[0].shape[1:]
        concat_in.append(
            jax.make_array_from_single_device_arrays(gshape, sh, shards))
    concat_zeros = [jnp.zeros((n_cores * s[0], *s[1:]), dt, device=sh)
                    for (s, dt) in zero_shapes]
    out_arrs = sharded(*concat_in, *concat_zeros)
    return [
        {name: np.asarray(out_arrs[i]).reshape(n_cores, *out_avals[i].shape)[c]
         for i, name in enumerate(out_names)}
        for c in range(n_cores)
    ]


def _run(nc, in_maps, core_ids):
    from concourse import bass_utils, bass2jax
    if not _TRACE:
        orig = bass2jax.run_bass_via_pjrt
        try:
            bass2jax.run_bass_via_pjrt = _patched_run_via_pjrt
            return bass_utils.run_bass_kernel_spmd(nc, in_maps, core_ids=core_ids)
        except Exception:
            pass
        finally:
            bass2jax.run_bass_via_pjrt = orig
    return bass_utils.run_bass_kernel_spmd(nc, in_maps, core_ids=core_ids,
                                           trace=_TRACE)


def _prep_weights(wi1, wh1, bi1, bh1, wi2, wh2, bi2, bh2,
                  w_init, b_init, w_out, b_out):
    f32 = np.float32
    w1t = np.ascontiguousarray(
        np.concatenate([wi1.T, wh1.T], 0)).astype(BF16).reshape(9, P, 3 * H)
    w2t = np.ascontiguousarray(
        np.concatenate([wi2.T, wh2.T], 0)).astype(BF16).reshape(16, P, 3 * H)
    wot = np.ascontiguousarray(w_out.T).astype(BF16).reshape(HK, P, P)
    wit = np.ascontiguousarray(w_init.T).astype(BF16)
    bias = np.zeros((P, 73), f32)
    bias[:, 0:16] = (bi1 + bh1)[:2048].reshape(16, P).T
    bias[:, 16:24] = bi1[2048:].reshape(8, P).T
    bias[:, 24:32] = bh1[2048:].reshape(8, P).T
    bias[:, 32:48] = (bi2 + bh2)[:2048].reshape(16, P).T
    bias[:, 48:56] = bi2[2048:].reshape(8, P).T
    bias[:, 56:64] = bh2[2048:].reshape(8, P).T
    bias[:, 64] = b_out
    bias[:, 65:73] = b_init.reshape(8, P).T
    return dict(w1t=w1t, w2t=w2t, wot=wot, wit=wit,
                bias=np.ascontiguousarray(bias))


def _prep_data(z, z8, rows):
    # zt: [P, S*N1*B]: (d, s*N1*B + t*B + b)
    zs = z[rows, :N1, :]                       # [CB, N1, D]
    zs = zs.reshape(S, B, N1, D).transpose(3, 0, 2, 1)   # [D, S, N1, B]
    ztp = np.ascontiguousarray(zs.reshape(D, S * N1 * B)).astype(BF16)
    z8s = z8[rows].reshape(S, B, D).transpose(2, 0, 1)   # [D, S, B]
    z8tp = np.ascontiguousarray(z8s.reshape(D, S * B)).astype(BF16)
    return dict(zt=ztp, z8t=z8tp)


def kernel(**inputs):
    n1 = int(inputs.get("n1", 16))
    assert n1 == N1, f"kernel hardcodes n1={N1}, got {n1}"
    tA = time.time()
    g = {k: np.asarray(v, dtype=np.float32) if k not in ("n1", "n2") else v
         for k, v in inputs.items()}

    wr = _prep_weights(g["wi1"], g["wh1"], g["bi1"], g["bh1"],
                       g["wi2"], g["wh2"], g["bi2"], g["bh2"],
                       g["w_init0"], g["b_init0"], g["w_out0"], g["b_out0"])
    wp = _prep_weights(g["wi3"], g["wh3"], g["bi3"], g["bh3"],
                       g["wi4"], g["wh4"], g["bi4"], g["bh4"],
                       g["w_init1"], g["b_init1"], g["w_out1"], g["b_out1"])

    in_maps = []
    for c in range(NCORES):
        grp, idx = (0, c) if c < G else (1, c - G)
        rows = slice(idx * CB, (idx + 1) * CB)
        if grp == 0:
            m = dict(wr, **_prep_data(g["zr"], g["zr8"], rows))
        else:
            m = dict(wp, **_prep_data(g["zp"], g["zp8"], rows))
        in_maps.append(m)

    tB = time.time()
    nc = _get_prog()
    t0 = time.time()
    res = _run(nc, in_maps, core_ids=list(range(NCORES)))
    _last["run_s"] = time.time() - t0
    _last["prep_s"] = tB - tA
    _last["build_s"] = t0 - tB
    _last["exec_time_ns"] = res.exec_time_ns

    def unpack(o):
        # [P, S*TOUT*B] -> [CB, TOUT, D]
        o = np.asarray(o, dtype=np.float32).reshape(D, S, TOUT, B)
        return np.ascontiguousarray(o.transpose(1, 3, 2, 0).reshape(CB, TOUT, D))

    outs = [unpack(r["out"]) for r in res.results]
    z_r = np.concatenate(outs[:G], axis=0)
    z_p = np.concatenate(outs[G:], axis=0)
    return z_p, z_r


# revision 12
# speedup vs baseline: 2.5073x; 2.5073x over previous
"""Trainium2 Bass kernel for the dual-GRU-decoder ("Interpolation") problem.

Strategy
--------
Two independent decoders (r: cells 1/2, p: cells 3/4). Each decoder is a
64-step GRU recurrence with B=2048, H=1024, D=128, n1=16.

The per-call wall clock on the axon path is dominated by (a) shipping the
NEFF + per-core inputs through the tunnel and (b) loading the NEFF; device
execution itself is ~10ms. So the kernel is built to minimize BYTES moved
per call:
  * hardware For_i loops over timesteps (static program ~10x smaller than
    full unrolling),
  * bf16 outputs (halves output upload/download; also halves the donated
    zero-buffer upload),
  * NCORES knob: with NCORES=2, each decoder runs on ONE core which
    processes the 2048 batch as 4 sequential slices of 512 — weights are
    then uploaded once per decoder instead of once per core (158MB -> 40MB).

Within a core, all weights are cast to bf16 and kept resident in SBUF
(~154 KiB/partition). Activations live in a transposed layout (feature dim
on partitions, batch on the free dim); the host pre/post-transposes.

Per step and per output chunk i (128 gate channels) the kernel accumulates
r/z gates over the concatenated [x; h] contraction in a single PSUM bank,
keeps the n-gate's input/hidden parts separate (r multiplies only the
hidden part), and applies sigmoid/tanh on the scalar engine with fused
per-partition biases. Hidden state is double-buffered (ping-pong); the
timestep loops are 2-step bodies so the ping-pong stays static.
"""

import time

import numpy as np
import ml_dtypes

BF16 = ml_dtypes.bfloat16
B_FULL, T, D, H, N1 = 2048, 64, 128, 1024, 16
TOUT = T - N1 + 1  # 49
HK = H // 128      # 8 hidden chunks
B = 512            # batch per slice (one matmul free-dim)
P = 128

NCORES = 2         # 2, 4, or 8; G = NCORES//2 cores per decoder
G = NCORES // 2
S = 4 // G         # sequential 512-slices per core
CB = S * B         # batch rows per core

_PROG = None
_TRACE = False
_last = {}


def _build_program():
    import concourse.mybir as mybir
    import concourse.tile as tile
    from concourse import bacc
    from concourse.bass import ds

    f32, bf16 = mybir.dt.float32, mybir.dt.bfloat16
    A = mybir.ActivationFunctionType
    # Bacc (not raw Bass): its compile() pass splits multi-semaphore waits
    # into event-semaphore trees — TRN2 allows at most 1 wait per instruction.
    nc = bacc.Bacc(None, target_bir_lowering=False)

    w1t = nc.dram_tensor("w1t", [9, P, 3 * H], bf16, kind="ExternalInput")
    w2t = nc.dram_tensor("w2t", [16, P, 3 * H], bf16, kind="ExternalInput")
    wot = nc.dram_tensor("wot", [HK, P, P], bf16, kind="ExternalInput")
    wit = nc.dram_tensor("wit", [P, H], bf16, kind="ExternalInput")
    bias = nc.dram_tensor("bias", [P, 73], f32, kind="ExternalInput")
    zt = nc.dram_tensor("zt", [P, S * N1 * B], bf16, kind="ExternalInput")
    z8t = nc.dram_tensor("z8t", [P, S * B], bf16, kind="ExternalInput")
    out_d = nc.dram_tensor("out", [P, S * TOUT * B], bf16, kind="ExternalOutput")

    with tile.TileContext(nc) as tc:
        with (
            tc.tile_pool(name="w", bufs=1) as wpool,
            tc.tile_pool(name="st", bufs=1) as spool,
            tc.tile_pool(name="zin", bufs=2) as zpool,
            tc.tile_pool(name="rz", bufs=2) as rzpool,
            tc.tile_pool(name="tmp", bufs=4) as tpool,
            tc.tile_pool(name="ost", bufs=1) as opool,
            tc.tile_pool(name="psum", bufs=8, space="PSUM") as ppool,
        ):
            # ---- resident weights ----
            w1 = wpool.tile([P, 9, 3 * H], bf16, tag="w1")
            for k in range(9):
                nc.sync.dma_start(w1[:, k, :], w1t[k])
            w2 = wpool.tile([P, 16, 3 * H], bf16, tag="w2")
            for k in range(16):
                nc.sync.dma_start(w2[:, k, :], w2t[k])
            wo = wpool.tile([P, HK, P], bf16, tag="wo")
            nc.sync.dma_start(wo[:], wot.rearrange("o p f -> p o f"))
            witl = wpool.tile([P, H], bf16, tag="wit")
            nc.sync.dma_start(witl[:], wit[:])
            bia = wpool.tile([P, 73], f32, tag="bias")
            nc.sync.dma_start(bia[:], bias[:])
            brz1, bni1, bnh1 = bia[:, 0:16], bia[:, 16:24], bia[:, 24:32]
            brz2, bni2, bnh2 = bia[:, 32:48], bia[:, 48:56], bia[:, 56:64]
            bout, bini = bia[:, 64:65], bia[:, 65:73]

            # ---- state (ping-pong) ----
            h0b = [spool.tile([P, HK, B], bf16, tag=f"h0{i}", name=f"h0{i}")
                   for i in range(2)]
            h1b = [spool.tile([P, HK, B], bf16, tag=f"h1{i}", name=f"h1{i}")
                   for i in range(2)]
            # feedback/output tiles: even steps write otE, odd steps write otO
            otE = spool.tile([P, B], bf16, tag="otE", name="otE")
            otO = spool.tile([P, B], bf16, tag="otO", name="otO")

            # consolidate the many init-DMA queue semaphores into one sync
            # point; otherwise downstream instructions exceed the per-inst
            # sync-wait slot limit in codegen.
            tc.strict_bb_all_engine_barrier()

            def gru_cell(w, rz_ks, in_ks, hn_ks, brz, bni, bnh, h_read, h_write):
                """One GRU cell step, transposed layout.

                rz_ks/in_ks/hn_ks: lists of (w_chunk_index, rhs_ap[128,B])
                pairs for the r/z accumulation, the n-gate input part, and
                the n-gate hidden part respectively.
                """
                for i in range(HK):
                    pr = ppool.tile([P, B], f32, tag="acc")
                    pz = ppool.tile([P, B], f32, tag="acc")
                    phn = ppool.tile([P, B], f32, tag="acc")
                    pin = ppool.tile([P, B], f32, tag="acc")
                    nrz = len(rz_ks)
                    for j, (k, rhs) in enumerate(rz_ks):
                        nc.tensor.matmul(pr[:], w[:, k, ds(i * P, P)], rhs,
                                         start=(j == 0), stop=(j == nrz - 1))
                    for j, (k, rhs) in enumerate(rz_ks):
                        nc.tensor.matmul(pz[:], w[:, k, ds((HK + i) * P, P)], rhs,
                                         start=(j == 0), stop=(j == nrz - 1))
                    for j, (k, rhs) in enumerate(hn_ks):
                        nc.tensor.matmul(phn[:], w[:, k, ds((2 * HK + i) * P, P)], rhs,
                                         start=(j == 0), stop=(j == len(hn_ks) - 1))
                    for j, (k, rhs) in enumerate(in_ks):
                        nc.tensor.matmul(pin[:], w[:, k, ds((2 * HK + i) * P, P)], rhs,
                                         start=(j == 0), stop=(j == len(in_ks) - 1))
                    r = rzpool.tile([P, B], bf16, tag="r")
                    zz = rzpool.tile([P, B], bf16, tag="z")
                    nc.scalar.activation(r[:], pr[:], A.Sigmoid, bias=brz[:, i:i + 1])
                    nc.scalar.activation(zz[:], pz[:], A.Sigmoid,
                                         bias=brz[:, HK + i:HK + i + 1])
                    a = tpool.tile([P, B], f32, tag="tmp")
                    nt = tpool.tile([P, B], f32, tag="tmp")
                    nc.scalar.add(a[:], phn[:], bnh[:, i:i + 1])   # h_n + b_hn
                    nc.vector.tensor_mul(a[:], r[:], a[:])         # r * (...)
                    nc.vector.tensor_add(a[:], a[:], pin[:])       # + i_n
                    nc.scalar.activation(nt[:], a[:], A.Tanh, bias=bni[:, i:i + 1])
                    nc.vector.tensor_sub(a[:], h_read[:, i, :], nt[:])  # h - n
                    nc.vector.tensor_mul(a[:], zz[:], a[:])             # z*(h-n)
                    nc.vector.tensor_add(h_write[:, i, :], nt[:], a[:])  # n + z*(h-n)

            def step(t, xT_ap, out_slice, ot_write, t0=False):
                """One GRU step at parity t%2. xT_ap: [P,B] input AP.
                out_slice: dram AP to store the step's output (or None).
                ot_write: bf16 [P,B] tile to hold the output (or None)."""
                par = t % 2
                h0r, h0w = h0b[par], h0b[1 - par]
                rz1 = [(1 + k, h0r[:, k, :]) for k in range(HK)] + [(0, xT_ap)]
                gru_cell(w1, rz1, [(0, xT_ap)],
                         [(1 + k, h0r[:, k, :]) for k in range(HK)],
                         brz1, bni1, bnh1, h0r, h0w)

                h1r = h0w if t0 else h1b[par]
                h1w = h1b[1 - par]
                rz2 = ([(8 + k, h1r[:, k, :]) for k in range(HK)]
                       + [(k, h0w[:, k, :]) for k in range(HK)])
                gru_cell(w2, rz2, [(k, h0w[:, k, :]) for k in range(HK)],
                         [(8 + k, h1r[:, k, :]) for k in range(HK)],
                         brz2, bni2, bnh2, h1r, h1w)

                if ot_write is not None:
                    po = ppool.tile([P, B], f32, tag="acc")
                    for k in range(HK):
                        nc.tensor.matmul(po[:], wo[:, k, :], h1w[:, k, :],
                                         start=(k == 0), stop=(k == HK - 1))
                    nc.scalar.add(ot_write[:], po[:], bout[:, 0:1])
                    if out_slice is not None:
                        nc.sync.dma_start(out_slice, ot_write[:])

            def load_x(off):
                xT = zpool.tile([P, B], bf16, tag="zin")
                nc.sync.dma_start(xT[:], zt[:, ds(off, B)])
                return xT

            def slice_body(s):
                zoff = s * (N1 * B)
                ooff = s * (TOUT * B)
                # ---- h0 init: h0 = z8 @ w_init.T + b_init ----
                z8l = zpool.tile([P, B], bf16, tag="zin")
                nc.sync.dma_start(z8l[:], z8t[:, ds(s * B, B)])
                for m in range(HK):
                    ps = ppool.tile([P, B], f32, tag="acc")
                    nc.tensor.matmul(ps[:], witl[:, ds(m * P, P)], z8l[:],
                                     start=True, stop=True)
                    nc.scalar.activation(h0b[0][:, m, :], ps[:], A.Identity,
                                         bias=bini[:, m:m + 1])
                # t = 0 unrolled (h1 seeding)
                step(0, load_x(zoff)[:], None, None, t0=True)
                # t = 1..14 input phase (odd, even per body)
                with tc.For_i(0, 7) as i:
                    step(1, load_x(zoff + (i * 2 + 1) * B)[:], None, None)
                    step(0, load_x(zoff + (i * 2 + 2) * B)[:], None, None)
                # t = 15 (emits out j=0 and seeds otO)
                step(1, load_x(zoff + 15 * B)[:], out_d[:, ds(ooff, B)], otO)
                # t = 16..63 feedback phase
                with tc.For_i(8, 32) as i:
                    step(0, otO[:], out_d[:, ds(ooff + (i * 2 - 15) * B, B)], otE)
                    step(1, otE[:], out_d[:, ds(ooff + (i * 2 - 14) * B, B)], otO)

            if S == 1:
                slice_body(0)
            else:
                with tc.For_i(0, S) as s:
                    slice_body(s)

    # Run Bacc's compile passes (register allocation, event-semaphore wait
    # splitting) before the module is serialized for the compiler.
    nc.finalize()
    return nc


def _get_prog():
    global _PROG
    if _PROG is None:
        _PROG = _build_program()
    return _PROG


def _patched_run_via_pjrt(nc, in_maps, n_cores):
    """Drop-in for bass2jax.run_bass_via_pjrt (multi-core, no-debug case)
    with two transfer optimizations:
      * donated zero output buffers are created device-side (jnp.zeros with
        a NamedSharding) instead of being uploaded as 50MB of host zeros;
      * per-core inputs are device_put per device and assembled with
        make_array_from_single_device_arrays (no host-side concat pass).
    Semantics are identical; outputs verified bit-equal to the stock path.
    """
    import jax
    import jax.numpy as jnp
    import numpy as np
    from jax.sharding import Mesh, NamedSharding, PartitionSpec
    from jax.experimental.shard_map import shard_map
    import concourse.mybir as mybir
    import concourse.bass2jax as b2j

    if nc.dbg_addr is not None or n_cores < 2:
        raise RuntimeError("unsupported; use stock path")
    b2j.install_neuronx_cc_hook()
    partition_name = nc.partition_id_tensor.name if nc.partition_id_tensor else None
    in_names, out_names, out_avals, zero_shapes = [], [], [], []
    for alloc in nc.m.functions[0].allocations:
        if not isinstance(alloc, mybir.MemoryLocationSet):
            continue
        name = alloc.memorylocations[0].name
        if alloc.kind == "ExternalInput":
            if name != partition_name:
                in_names.append(name)
        elif alloc.kind == "ExternalOutput":
            shape = tuple(alloc.tensor_shape)
            out_names.append(name)
            out_avals.append(jax.core.ShapedArray(shape, mybir.dt.np(alloc.dtype)))
            zero_shapes.append((shape, mybir.dt.np(alloc.dtype)))
    n_params = len(in_names)
    n_outs = len(out_avals)
    in_names_full = in_names + out_names
    if partition_name is not None:
        in_names_full.append(partition_name)
    donate = tuple(range(n_params, n_params + n_outs))

    def _body(*args):
        operands = list(args)
        if partition_name is not None:
            operands.append(b2j.partition_id_tensor())
        outs = b2j._bass_exec_p.bind(
            *operands,
            out_avals=tuple(out_avals),
            in_names=tuple(in_names_full),
            out_names=tuple(out_names),
            lowering_input_output_aliases=(),
            sim_require_finite=True,
            sim_require_nnan=True,
            nc=nc,
        )
        return tuple(outs)

    devices = jax.devices()[:n_cores]
    assert len(devices) == n_cores
    mesh = Mesh(np.asarray(devices), ("core",))
    in_specs = (PartitionSpec("core"),) * (n_params + n_outs)
    out_specs = (PartitionSpec("core"),) * len(out_names)
    sharded = jax.jit(
        shard_map(_body, mesh=mesh, in_specs=in_specs, out_specs=out_specs,
                  check_rep=False),
        donate_argnums=donate, keep_unused=True,
    )
    sh = NamedSharding(mesh, PartitionSpec("core"))
    per_core = [[np.asarray(m[name]) for name in in_names] for m in in_maps]
    concat_in = []
    for i in range(n_params):
        shards = [jax.device_put(per_core[c][i], devices[c])
                  for c in range(n_cores)]
        gshape = (n_cores * shards[0].shape[0],) + shards[0].shape[1:]
        concat_in.append(
            jax.make_array_from_single_device_arrays(gshape, sh, shards))
    concat_zeros = [jnp.zeros((n_cores * s[0], *s[1:]), dt, device=sh)
                    for (s, dt) in zero_shapes]
    out_arrs = sharded(*concat_in, *concat_zeros)
    return [
        {name: np.asarray(out_arrs[i]).reshape(n_cores, *out_avals[i].shape)[c]
         for i, name in enumerate(out_names)}
        for c in range(n_cores)
    ]


def _run(nc, in_maps, core_ids):
    from concourse import bass_utils, bass2jax
    if not _TRACE:
        orig = bass2jax.run_bass_via_pjrt
        try:
            bass2jax.run_bass_via_pjrt = _patched_run_via_pjrt
            return bass_utils.run_bass_kernel_spmd(nc, in_maps, core_ids=core_ids)
        except Exception:
            pass
        finally:
            bass2jax.run_bass_via_pjrt = orig
    return bass_utils.run_bass_kernel_spmd(nc, in_maps, core_ids=core_ids,
                                           trace=_TRACE)


def _prep_weights(wi1, wh1, bi1, bh1, wi2, wh2, bi2, bh2,
                  w_init, b_init, w_out, b_out):
    f32 = np.float32
    w1t = np.empty((9 * P, 3 * H), BF16)
    w1t[:D] = wi1.T
    w1t[D:] = wh1.T
    w1t = w1t.reshape(9, P, 3 * H)
    w2t = np.empty((16 * P, 3 * H), BF16)
    w2t[:H] = wi2.T
    w2t[H:] = wh2.T
    w2t = w2t.reshape(16, P, 3 * H)
    wot = np.ascontiguousarray(w_out.T).astype(BF16).reshape(HK, P, P)
    wit = np.ascontiguousarray(w_init.T).astype(BF16)
    bias = np.zeros((P, 73), f32)
    bias[:, 0:16] = (bi1 + bh1)[:2048].reshape(16, P).T
    bias[:, 16:24] = bi1[2048:].reshape(8, P).T
    bias[:, 24:32] = bh1[2048:].reshape(8, P).T
    bias[:, 32:48] = (bi2 + bh2)[:2048].reshape(16, P).T
    bias[:, 48:56] = bi2[2048:].reshape(8, P).T
    bias[:, 56:64] = bh2[2048:].reshape(8, P).T
    bias[:, 64] = b_out
    bias[:, 65:73] = b_init.reshape(8, P).T
    return dict(w1t=w1t, w2t=w2t, wot=wot, wit=wit,
                bias=np.ascontiguousarray(bias))


def _prep_data(z, z8, rows):
    # zt: [P, S*N1*B]: (d, s*N1*B + t*B + b)
    zs = z[rows, :N1, :]                       # [CB, N1, D]
    zs = zs.reshape(S, B, N1, D).transpose(3, 0, 2, 1)   # [D, S, N1, B]
    ztp = np.ascontiguousarray(zs.reshape(D, S * N1 * B)).astype(BF16)
    z8s = z8[rows].reshape(S, B, D).transpose(2, 0, 1)   # [D, S, B]
    z8tp = np.ascontiguousarray(z8s.reshape(D, S * B)).astype(BF16)
    return dict(zt=ztp, z8t=z8tp)


def kernel(**inputs):
    n1 = int(inputs.get("n1", 16))
    assert n1 == N1, f"kernel hardcodes n1={N1}, got {n1}"
    tA = time.time()
    g = {k: np.asarray(v, dtype=np.float32) if k not in ("n1", "n2") else v
         for k, v in inputs.items()}

    wr = _prep_weights(g["wi1"], g["wh1"], g["bi1"], g["bh1"],
                       g["wi2"], g["wh2"], g["bi2"], g["bh2"],
                       g["w_init0"], g["b_init0"], g["w_out0"], g["b_out0"])
    wp = _prep_weights(g["wi3"], g["wh3"], g["bi3"], g["bh3"],
                       g["wi4"], g["wh4"], g["bi4"], g["bh4"],
                       g["w_init1"], g["b_init1"], g["w_out1"], g["b_out1"])

    in_maps = []
    for c in range(NCORES):
        grp, idx = (0, c) if c < G else (1, c - G)
        rows = slice(idx * CB, (idx + 1) * CB)
        if grp == 0:
            m = dict(wr, **_prep_data(g["zr"], g["zr8"], rows))
        else:
            m = dict(wp, **_prep_data(g["zp"], g["zp8"], rows))
        in_maps.append(m)

    tB = time.time()
    nc = _get_prog()
    t0 = time.time()
    res = _run(nc, in_maps, core_ids=list(range(NCORES)))
    _last["run_s"] = time.time() - t0
    _last["prep_s"] = tB - tA
    _last["build_s"] = t0 - tB
    _last["exec_time_ns"] = res.exec_time_ns

    def unpack(o):
        # [P, S*TOUT*B] -> [CB, TOUT, D]
        o = np.asarray(o, dtype=np.float32).reshape(D, S, TOUT, B)
        return np.ascontiguousarray(o.transpose(1, 3, 2, 0).reshape(CB, TOUT, D))

    outs = [unpack(r["out"]) for r in res.results]
    z_r = np.concatenate(outs[:G], axis=0)
    z_p = np.concatenate(outs[G:], axis=0)
    return z_p, z_r
